# revision 12
# baseline (speedup 1.0000x reference)
"""Trainium2 Bass kernel for the NSDE model (Euler-Maruyama scan + MLPs).

Strategy:
  - Data-parallel over batch: 16384 rows -> 8 cores x 2048 rows.
  - Only the 20 time slices of x_path that the scan actually reads are
    shipped to the device (indices computed on host from t_span).
  - Feature-major layout on chip: activations are [feature, batch] so every
    matmul uses the weight matrix directly as lhsT (out = W^T @ actT), and
    biases are per-partition scalars.
  - 64-feature tensors (h, x, zs, g1, sigmoid, drift-out) are "packed":
    partitions 0-63 hold features of batch half A (rows 0..1023), partitions
    64-127 hold features of batch half B (rows 1024..2047). The drift layer1
    uses PE row tiling (tile_position=(64,0)) with duplicated weights; the
    diffusion layers use block-diagonal weights; the drift output uses
    column tiling (A -> psum rows 0:64, B -> rows 64:128 via (0,64)).
  - Bulk matmuls run in bf16 (1 cycle/row, N=1024). The h carry stays high
    precision: h is stored as float32r, added into the drift-out PSUM with a
    float32r identity matmul (f32r streams ~1.5 cyc/row; fp32 would lower
    to two half-rate passes). A bf16 working copy h_bf feeds bulk matmuls.
  - h' = (psum3 + dt*db3) + sigmoid*zs via one fused scalar_tensor_tensor.
  - dt and sqrt(dt) folds are done on host: dW3*dt per step, db3*dt, and
    the noise is pre-scaled zs = dW * gscale * sqrt(dt) (bf16).
"""

import os
from contextlib import ExitStack

import ml_dtypes
import numpy as np

import concourse.bass as bass
import concourse.mybir as mybir
import concourse.tile as tile
from concourse import bacc
from concourse.bass_utils import run_bass_kernel_spmd

F32 = mybir.dt.float32
F32R = mybir.dt.float32r
BF16 = mybir.dt.bfloat16
AF = mybir.ActivationFunctionType
ALU = mybir.AluOpType

NPBF = ml_dtypes.bfloat16

STEPS = 20
NCORES = 8
B = 16384
BC = B // NCORES  # per-core batch: 2048
HALF = BC // 2  # packed half: 1024
H = 64  # hidden state size
FX = 64  # x feature size
DW = 128  # drift MLP width

_CACHE = {}


def _build():
    if "nc" in _CACHE:
        return _CACHE["nc"]

    nc = bacc.Bacc("TRN2", target_bir_lowering=False, debug=False)

    def din(name, shape, dt=F32):
        return nc.dram_tensor(name, shape, dt, kind="ExternalInput")

    d_xt = din("xt", [STEPS, 128, HALF], BF16)  # packed x slices (feature-major)
    d_zst = din("zst", [STEPS, 128, HALF], BF16)  # packed gscale*sqrt(dt)*dW
    d_w1h = din("w1h", [128, DW], BF16)  # dW1[:64] duplicated on partitions 64-127
    d_w1x = din("w1x", [128, DW], BF16)  # dW1[64:] duplicated
    d_w2 = din("w2", [DW, DW], BF16)
    d_w3s = din("w3s", [STEPS, DW, H], BF16)  # dW3 * dt_k
    d_gw1 = din("gw1", [128, DW], BF16)  # blockdiag(gW1, gW1)
    d_gw2 = din("gw2", [128, DW], BF16)  # blockdiag(gW2, gW2)
    d_id = din("ident", [128, DW], F32R)  # 128x128 identity (f32r h carry)
    d_b1 = din("b1", [DW, 1])
    d_b2 = din("b2", [DW, 1])
    d_dtb3 = din("dtb3", [128, STEPS])  # dt_k * db3, packed-duplicated
    d_gb1 = din("gb1", [128, 1])  # gb1 duplicated
    d_gb2 = din("gb2", [128, 1])  # gb2 duplicated
    d_rw1 = din("rw1", [H, 32], BF16)
    d_rb1 = din("rb1", [32, 1])
    d_rw2 = din("rw2", [32, 2], BF16)
    d_rb2 = din("rb2", [2, 1])
    d_h0 = din("h0", [128, HALF], F32R)  # zeros (f32r memset unsupported)
    d_h0b = din("h0b", [128, HALF], BF16)  # zeros
    d_out = nc.dram_tensor("out", [2, BC], F32, kind="ExternalOutput")

    with ExitStack() as ctx:
        tc = ctx.enter_context(tile.TileContext(nc))
        consts = ctx.enter_context(tc.tile_pool(name="consts", bufs=1))
        xzp = ctx.enter_context(tc.tile_pool(name="xzp", bufs=3))
        hp = ctx.enter_context(tc.tile_pool(name="hp", bufs=2))
        wk = ctx.enter_context(tc.tile_pool(name="wk", bufs=2))
        ppb = ctx.enter_context(tc.tile_pool(name="ppb", bufs=1, space="PSUM"))
        pps = ctx.enter_context(tc.tile_pool(name="pps", bufs=2, space="PSUM"))

        def cload(dram_ap, shape, name, dt=F32):
            t = consts.tile(shape, dt, name=name, tag=name)
            nc.sync.dma_start(t[:], dram_ap)
            return t

        w1h = cload(d_w1h[:, :], [128, DW], "w1h", BF16)
        w1x = cload(d_w1x[:, :], [128, DW], "w1x", BF16)
        w2 = cload(d_w2[:, :], [DW, DW], "w2", BF16)
        w3s = cload(
            d_w3s[:, :, :].rearrange("k p m -> p k m"), [DW, STEPS, H], "w3s", BF16
        )
        gw1 = cload(d_gw1[:, :], [128, DW], "gw1", BF16)
        gw2 = cload(d_gw2[:, :], [128, DW], "gw2", BF16)
        idn = cload(d_id[:, :], [128, DW], "idn", F32R)
        b1 = cload(d_b1[:, :], [DW, 1], "b1")
        b2 = cload(d_b2[:, :], [DW, 1], "b2")
        dtb3 = cload(d_dtb3[:, :], [128, STEPS], "dtb3")
        gb1 = cload(d_gb1[:, :], [128, 1], "gb1")
        gb2 = cload(d_gb2[:, :], [128, 1], "gb2")
        rw1 = cload(d_rw1[:, :], [H, 32], "rw1", BF16)
        rb1 = cload(d_rb1[:, :], [32, 1], "rb1")
        rw2 = cload(d_rw2[:, :], [32, 2], "rw2", BF16)
        rb2 = cload(d_rb2[:, :], [2, 1], "rb2")

        h_cur = hp.tile([128, HALF], F32R, name="h", tag="h")
        nc.sync.dma_start(h_cur[:], d_h0[:, :])
        hb_cur = hp.tile([128, HALF], BF16, name="hb", tag="hb")
        nc.sync.dma_start(hb_cur[:], d_h0b[:, :])

        for k in range(STEPS):
            xk = xzp.tile([128, HALF], BF16, name="xk", tag="xk")
            nc.sync.dma_start(xk[:], d_xt[k])
            zk = xzp.tile([128, HALF], BF16, name="zk", tag="zk")
            nc.sync.dma_start(zk[:], d_zst[k])

            # ---- diffusion branch (packed, blockdiag weights, N=1024) ----
            psg = pps.tile([128, HALF], F32, name="psg", tag="pps")
            for j in range(2):
                sl = slice(j * 512, (j + 1) * 512)
                nc.tensor.matmul(
                    psg[:, sl], gw1[:, :], hb_cur[:, sl], start=True, stop=True
                )
            g1 = wk.tile([128, HALF], BF16, name="g1", tag="g1")
            nc.scalar.activation(g1[:], psg[:], AF.Relu, bias=gb1[:])

            pss = pps.tile([128, HALF], F32, name="pss", tag="pps")
            for j in range(2):
                sl = slice(j * 512, (j + 1) * 512)
                nc.tensor.matmul(
                    pss[:, sl], gw2[:, :], g1[:, sl], start=True, stop=True
                )
            sg = wk.tile([128, HALF], BF16, name="sg", tag="sg")
            nc.scalar.activation(sg[:], pss[:], AF.Sigmoid, bias=gb2[:])

            # noise term t = sigmoid * zs on GPSIMD (SBUF-only op)
            tt = wk.tile([128, HALF], BF16, name="tt", tag="tt")
            nc.gpsimd.tensor_mul(tt[:], sg[:], zk[:])

            # ---- drift layer 1: z1 = relu(W1h^T h + W1x^T x + b1) ----
            # [128 feats, 2048 batch]; batch half A <- packed rows 0-63,
            # half B <- rows 64-127 (concurrent PE row groups).
            ps1 = ppb.tile([128, 2 * HALF], F32, name="ps1", tag="ppb")
            for j in range(2):
                sl = slice(j * 512, (j + 1) * 512)
                slh = slice(HALF + j * 512, HALF + (j + 1) * 512)
                nc.tensor.matmul(
                    ps1[:, sl], w1h[0:64, :], hb_cur[0:64, sl],
                    start=True, stop=False,
                )
                nc.tensor.matmul(
                    ps1[:, sl], w1x[0:64, :], xk[0:64, sl],
                    start=False, stop=True,
                )
                nc.tensor.matmul(
                    ps1[:, slh], w1h[64:128, :], hb_cur[64:128, sl],
                    start=True, stop=False, tile_position=(64, 0),
                )
                nc.tensor.matmul(
                    ps1[:, slh], w1x[64:128, :], xk[64:128, sl],
                    start=False, stop=True, tile_position=(64, 0),
                )
            z1 = wk.tile([128, 2 * HALF], BF16, name="z1", tag="z1")
            nc.scalar.activation(z1[:], ps1[:], AF.Relu, bias=b1[:])

            # ---- drift layer 2: z2 = relu(W2^T z1 + b2) ----
            ps2 = ppb.tile([128, 2 * HALF], F32, name="ps2", tag="ppb")
            for j in range(4):
                sl = slice(j * 512, (j + 1) * 512)
                nc.tensor.matmul(
                    ps2[:, sl], w2[:, :], z1[:, sl], start=True, stop=True
                )
            z2 = wk.tile([128, 2 * HALF], BF16, name="z2", tag="z2")
            nc.vector.tensor_scalar(z2[:], ps2[:], b2[:], 0.0, ALU.add, ALU.max)

            # ---- drift out + h carry: ps3 = dt*(z2 @ dW3) + h ----
            # col tiling: half A -> psum rows 0:64 via (0,0), half B ->
            # rows 64:128 via (0,64); h added exactly (f32r identity matmul).
            ps3 = pps.tile([128, HALF], F32, name="ps3", tag="pps")
            for j in range(2):
                sl = slice(j * 512, (j + 1) * 512)
                slh = slice(HALF + j * 512, HALF + (j + 1) * 512)
                nc.tensor.matmul(
                    ps3[0:64, sl], w3s[:, k, :], z2[:, sl],
                    start=True, stop=False, skip_group_check=True,
                )
                nc.tensor.matmul(
                    ps3[64:128, sl], w3s[:, k, :], z2[:, slh],
                    start=True, stop=False, tile_position=(0, 64),
                    skip_group_check=True,
                )
                nc.tensor.matmul(
                    ps3[:, sl], idn[:, :], h_cur[:, sl],
                    start=False, stop=True, skip_group_check=True,
                )
            # h' = (ps3 + dt*db3) + sigmoid*zs ; bf16 working copy for matmuls
            h_new = hp.tile([128, HALF], F32R, name="h", tag="h")
            nc.vector.scalar_tensor_tensor(
                h_new[:], ps3[:], dtb3[:, k : k + 1], tt[:], ALU.add, ALU.add
            )
            hb_new = hp.tile([128, HALF], BF16, name="hb", tag="hb")
            nc.vector.tensor_copy(hb_new[:], h_new[:])
            h_cur = h_new
            hb_cur = hb_new

        # ---- readout: out = relu(h @ rW1 + rb1) @ rW2 + rb2 ----
        h_unp = wk.tile([H, BC], BF16, name="h_unp", tag="h_unp")
        nc.sync.dma_start(h_unp[:, 0:HALF], hb_cur[0:64, :])
        nc.sync.dma_start(h_unp[:, HALF:], hb_cur[64:128, :])

        r1 = wk.tile([32, BC], BF16, name="r1", tag="r1")
        for half in range(2):
            sl = slice(half * HALF, (half + 1) * HALF)
            psr = pps.tile([128, HALF], F32, name="psr", tag="pps")
            for j in range(2):
                sj = slice(j * 512, (j + 1) * 512)
                sij = slice(half * HALF + j * 512, half * HALF + (j + 1) * 512)
                nc.tensor.matmul(
                    psr[0:32, sj], rw1[:, :], h_unp[:, sij], start=True, stop=True
                )
            nc.scalar.activation(r1[:, sl], psr[0:32, :], AF.Relu, bias=rb1[:])

        osb = wk.tile([2, BC], F32, name="osb", tag="osb")
        for half in range(2):
            sl = slice(half * HALF, (half + 1) * HALF)
            pso = pps.tile([128, HALF], F32, name="pso", tag="pps")
            for j in range(2):
                sj = slice(j * 512, (j + 1) * 512)
                sij = slice(half * HALF + j * 512, half * HALF + (j + 1) * 512)
                nc.tensor.matmul(
                    pso[0:2, sj], rw2[:, :], r1[:, sij], start=True, stop=True
                )
            nc.scalar.activation(
                osb[:, sl], pso[0:2, :], AF.Identity, bias=rb2[:]
            )
        nc.sync.dma_start(d_out[:, :], osb[:])

    nc.compile()
    _CACHE["nc"] = nc
    return nc


def _dup(a, dt=NPBF):
    return np.ascontiguousarray(np.concatenate([a, a], axis=0).astype(dt))


def _blkdiag(a, dt=NPBF):
    n, m = a.shape
    out = np.zeros((2 * n, 2 * m), np.float32)
    out[:n, :m] = a
    out[n:, m:] = a
    return np.ascontiguousarray(out.astype(dt))


def _prep_in_maps(inputs):
    xp = np.asarray(inputs["x_path"], dtype=np.float32)
    t_span = np.asarray(inputs["t_span"], dtype=np.float32)
    dw = np.asarray(inputs["dW"], dtype=np.float32)

    Tm1 = np.int32(xp.shape[1] - 1)
    t_max = t_span[-1]
    idx = np.clip(
        (t_span[:-1] / t_max * np.float32(Tm1)).astype(np.int32), 0, Tm1
    )
    dts = (t_span[1:] - t_span[:-1]).astype(np.float32)
    sq = np.sqrt(dts).astype(np.float32)

    gscale = np.asarray(inputs["gscale"], dtype=np.float32)
    w1 = np.asarray(inputs["dW1"], dtype=np.float32)
    w2 = np.asarray(inputs["dW2"], dtype=np.float32)
    w3 = np.asarray(inputs["dW3"], dtype=np.float32)
    db1 = np.asarray(inputs["db1"], dtype=np.float32)
    db2 = np.asarray(inputs["db2"], dtype=np.float32)
    db3 = np.asarray(inputs["db3"], dtype=np.float32)
    gw1 = np.asarray(inputs["gW1"], dtype=np.float32)
    gw2 = np.asarray(inputs["gW2"], dtype=np.float32)
    gb1 = np.asarray(inputs["gb1"], dtype=np.float32)
    gb2 = np.asarray(inputs["gb2"], dtype=np.float32)
    rw1 = np.asarray(inputs["rW1"], dtype=np.float32)
    rb1 = np.asarray(inputs["rb1"], dtype=np.float32)
    rw2 = np.asarray(inputs["rW2"], dtype=np.float32)
    rb2 = np.asarray(inputs["rb2"], dtype=np.float32)

    w3s = w3[None, :, :] * dts[:, None, None]  # [STEPS, DW, H]

    common = {
        "w1h": _dup(w1[:H]),
        "w1x": _dup(w1[H:]),
        "w2": np.ascontiguousarray(w2.astype(NPBF)),
        "w3s": np.ascontiguousarray(w3s.astype(NPBF)),
        "gw1": _blkdiag(gw1),
        "gw2": _blkdiag(gw2),
        "ident": np.eye(DW, dtype=np.float32),
        "b1": np.ascontiguousarray(db1.reshape(DW, 1)),
        "b2": np.ascontiguousarray(db2.reshape(DW, 1)),
        "dtb3": _dup((dts[:, None] * db3[None, :]).T, np.float32),  # [128, STEPS]
        "gb1": _dup(gb1.reshape(H, 1), np.float32),
        "gb2": _dup(gb2.reshape(H, 1), np.float32),
        "rw1": np.ascontiguousarray(rw1.astype(NPBF)),
        "rb1": np.ascontiguousarray(rb1.reshape(32, 1)),
        "rw2": np.ascontiguousarray(rw2.astype(NPBF)),
        "rb2": np.ascontiguousarray(rb2.reshape(2, 1)),
        "h0": np.zeros((128, HALF), np.float32),
        "h0b": np.zeros((128, HALF), NPBF),
    }

    xg = xp[:, idx, :]  # [B, STEPS, F]
    zsc = gscale[None, :] * sq[:, None]  # [STEPS, F]

    in_maps = []
    for c in range(NCORES):
        rows = slice(c * BC, (c + 1) * BC)
        # x: (b2, b', k, f) -> (k, b2, f, b') -> [STEPS, 128, HALF]
        xt = np.ascontiguousarray(
            xg[rows]
            .reshape(2, HALF, STEPS, FX)
            .transpose(2, 0, 3, 1)
            .reshape(STEPS, 128, HALF)
            .astype(NPBF)
        )
        zc = dw[:, rows, :] * zsc[:, None, :]  # [STEPS, BC, H]
        zst = np.ascontiguousarray(
            zc.reshape(STEPS, 2, HALF, H)
            .transpose(0, 1, 3, 2)
            .reshape(STEPS, 128, HALF)
            .astype(NPBF)
        )
        m = dict(common)
        m["xt"] = xt
        m["zst"] = zst
        in_maps.append(m)
    return in_maps


def kernel(**inputs):
    nc = _build()
    in_maps = _prep_in_maps(inputs)
    run_kwargs = dict(_CACHE.get("run_kwargs", {}))
    res = run_bass_kernel_spmd(nc, in_maps, list(range(NCORES)), **run_kwargs)
    _CACHE["last_results"] = res
    mu = np.concatenate([res.results[c]["out"][0] for c in range(NCORES)])
    ls = np.concatenate([res.results[c]["out"][1] for c in range(NCORES)])
    return mu, ls


# revision 14
# speedup vs baseline: 1.0221x; 1.0221x over previous
"""Trainium2 Bass kernel for the NSDE model (Euler-Maruyama scan + MLPs).

Strategy:
  - Data-parallel over batch: 16384 rows -> 8 cores x 2048 rows.
  - Only the 20 time slices of x_path that the scan actually reads are
    shipped to the device (indices computed on host from t_span).
  - Feature-major layout on chip: activations are [feature, batch] so every
    matmul uses the weight matrix directly as lhsT (out = W^T @ actT) and
    biases are per-partition scalars.
  - The per-core batch (2048) is processed as TWO independent interleaved
    streams of 1024 rows. The Euler-Maruyama scan is inherently serial, so a
    single stream leaves every engine idle most of the time; two streams
    keep the tensor/scalar/vector engines busy with the other stream while
    one waits on its dependency chain.
  - Within a stream, 64-feature tensors are "packed": partitions 0-63 hold
    features of its first 512 rows, partitions 64-127 the second 512. Drift
    layer1 uses PE row tiling (tile_position=(64,0), duplicated weights);
    diffusion layers use block-diagonal weights; drift output uses column
    tiling ((0,64) writes psum rows 64:128).
  - Bulk matmuls are bf16 (1 cycle/row). The h carry stays high precision:
    h is float32r (4-byte, ~1 cyc/row vs fp32's two half-rate passes) and
    is added into the drift-out PSUM exactly via a f32r identity matmul.
    h's consumers (drift l1 h-part, diffusion l1, identity) run f32r.
  - h' = (psum3 + dt*db3) + sigmoid*zs via one fused scalar_tensor_tensor.
  - dt and sqrt(dt) folds are done on host: dW3*dt per step, db3*dt, and
    the noise is pre-scaled zs = dW * gscale * sqrt(dt) (bf16).
  - Elementwise work is balanced across ScalarE/VectorE (both ~1 elem/cyc
    from PSUM) with the noise multiply on GpSimd.
"""

import os
from contextlib import ExitStack

import ml_dtypes
import numpy as np

import concourse.bass as bass
import concourse.mybir as mybir
import concourse.tile as tile
from concourse import bacc
from concourse.bass_utils import run_bass_kernel_spmd

F32 = mybir.dt.float32
F32R = mybir.dt.float32r
BF16 = mybir.dt.bfloat16
AF = mybir.ActivationFunctionType
ALU = mybir.AluOpType

NPBF = ml_dtypes.bfloat16

STEPS = 20
NCORES = 8
B = 16384
BC = B // NCORES  # per-core batch: 2048
SB = BC // 2  # per-stream batch: 1024
HB = SB // 2  # packed free size per stream: 512
H = 64
FX = 64
DW = 128

_CACHE = {}


def _build():
    if "nc" in _CACHE:
        return _CACHE["nc"]

    nc = bacc.Bacc("TRN2", target_bir_lowering=False, debug=False)

    def din(name, shape, dt=F32):
        return nc.dram_tensor(name, shape, dt, kind="ExternalInput")

    d_xt = din("xt", [STEPS, 2, 128, HB], BF16)  # [step, stream, feat-packed, b]
    d_zst = din("zst", [STEPS, 2, 128, HB], BF16)
    d_w1h = din("w1h", [128, DW], F32R)  # dW1[:64] duplicated (f32r: reads h)
    d_w1x = din("w1x", [128, DW], BF16)  # dW1[64:] duplicated
    d_w2 = din("w2", [DW, DW], BF16)
    d_w3s = din("w3s", [STEPS, DW, H], BF16)  # dW3 * dt_k
    d_gw1 = din("gw1", [128, DW], F32R)  # blockdiag(gW1, gW1) (f32r: reads h)
    d_gw2 = din("gw2", [DW, DW], BF16)  # blockdiag(gW2, gW2)
    d_id = din("ident", [128, DW], F32R)  # 128x128 identity (h carry)
    d_b1 = din("b1", [DW, 1])
    d_b2 = din("b2", [DW, 1])
    d_dtb3 = din("dtb3", [128, STEPS])  # dt_k * db3, packed-dup
    d_gb1 = din("gb1", [128, 1])
    d_gb2 = din("gb2", [128, 1])
    d_rw1 = din("rw1", [H, 32], F32R)
    d_rb1 = din("rb1", [32, 1])
    d_rw2 = din("rw2", [32, 2], BF16)
    d_rb2 = din("rb2", [2, 1])
    d_h0 = din("h0", [128, HB], F32R)  # zeros (f32r memset unsupported)
    d_out = nc.dram_tensor("out", [2, BC], F32, kind="ExternalOutput")

    with ExitStack() as ctx:
        tc = ctx.enter_context(tile.TileContext(nc))
        consts = ctx.enter_context(tc.tile_pool(name="consts", bufs=1))
        xzp = ctx.enter_context(tc.tile_pool(name="xzp", bufs=3))
        hp = ctx.enter_context(tc.tile_pool(name="hp", bufs=2))
        wk = ctx.enter_context(tc.tile_pool(name="wk", bufs=2))
        ppb = ctx.enter_context(tc.tile_pool(name="ppb", bufs=2, space="PSUM"))
        pps = ctx.enter_context(tc.tile_pool(name="pps", bufs=4, space="PSUM"))

        def cload(dram_ap, shape, name, dt=F32):
            t = consts.tile(shape, dt, name=name, tag=name)
            nc.sync.dma_start(t[:], dram_ap)
            return t

        w1h = cload(d_w1h[:, :], [128, DW], "w1h", F32R)
        w1x = cload(d_w1x[:, :], [128, DW], "w1x", BF16)
        w2 = cload(d_w2[:, :], [DW, DW], "w2", BF16)
        w3s = cload(
            d_w3s[:, :, :].rearrange("k p m -> p k m"), [DW, STEPS, H], "w3s", BF16
        )
        gw1 = cload(d_gw1[:, :], [128, DW], "gw1", F32R)
        gw2 = cload(d_gw2[:, :], [DW, DW], "gw2", BF16)
        idn = cload(d_id[:, :], [128, DW], "idn", F32R)
        b1 = cload(d_b1[:, :], [DW, 1], "b1")
        b2 = cload(d_b2[:, :], [DW, 1], "b2")
        dtb3 = cload(d_dtb3[:, :], [128, STEPS], "dtb3")
        gb1 = cload(d_gb1[:, :], [128, 1], "gb1")
        gb2 = cload(d_gb2[:, :], [128, 1], "gb2")
        rw1 = cload(d_rw1[:, :], [H, 32], "rw1", F32R)
        rb1 = cload(d_rb1[:, :], [32, 1], "rb1")
        rw2 = cload(d_rw2[:, :], [32, 2], "rw2", BF16)
        rb2 = cload(d_rb2[:, :], [2, 1], "rb2")

        h_cur = []
        for s in range(2):
            h0 = hp.tile([128, HB], F32R, name=f"h{s}", tag=f"h{s}")
            nc.sync.dma_start(h0[:], d_h0[:, :])
            h_cur.append(h0)

        def step(k, s, h_s):
            """One Euler-Maruyama step for stream s; returns new h tile."""
            xk = xzp.tile([128, HB], BF16, name=f"xk{s}", tag=f"xk{s}")
            nc.sync.dma_start(xk[:], d_xt[k, s])
            zk = xzp.tile([128, HB], BF16, name=f"zk{s}", tag=f"zk{s}")
            nc.sync.dma_start(zk[:], d_zst[k, s])

            # ---- diffusion branch (blockdiag weights) ----
            psg = pps.tile([128, HB], F32, name=f"psg{s}", tag="pps")
            nc.tensor.matmul(psg[:, :], gw1[:, :], h_s[:, :], start=True, stop=True)
            g1 = wk.tile([128, HB], BF16, name=f"g1{s}", tag=f"g1{s}")
            if s == 0:
                nc.scalar.activation(g1[:], psg[:], AF.Relu, bias=gb1[:])
            else:
                nc.vector.tensor_scalar(g1[:], psg[:], gb1[:], 0.0, ALU.add, ALU.max)

            pss = pps.tile([128, HB], F32, name=f"pss{s}", tag="pps")
            nc.tensor.matmul(pss[:, :], gw2[:, :], g1[:, :], start=True, stop=True)
            sg = wk.tile([128, HB], BF16, name=f"sg{s}", tag=f"sg{s}")
            nc.scalar.activation(sg[:], pss[:], AF.Sigmoid, bias=gb2[:])

            # noise term t = sigmoid * zs
            tt = wk.tile([128, HB], BF16, name=f"tt{s}", tag=f"tt{s}")
            nc.gpsimd.tensor_mul(tt[:], sg[:], zk[:])

            # ---- drift layer 1: z1 = relu(W1h^T h + W1x^T x + b1) ----
            ps1 = ppb.tile([128, SB], F32, name=f"ps1{s}", tag="ppb")
            for j, tp in ((0, None), (1, (64, 0))):
                lo, hi = 64 * j, 64 * (j + 1)
                sl = slice(j * HB, (j + 1) * HB)
                nc.tensor.matmul(
                    ps1[:, sl], w1h[lo:hi, :], h_s[lo:hi, :],
                    start=True, stop=False, tile_position=tp,
                )
                nc.tensor.matmul(
                    ps1[:, sl], w1x[lo:hi, :], xk[lo:hi, :],
                    start=False, stop=True, tile_position=tp,
                )
            z1 = wk.tile([128, SB], BF16, name=f"z1{s}", tag=f"z1{s}")
            nc.scalar.activation(z1[:, 0:HB], ps1[:, 0:HB], AF.Relu, bias=b1[:])
            nc.vector.tensor_scalar(
                z1[:, HB:], ps1[:, HB:], b1[:], 0.0, ALU.add, ALU.max
            )

            # ---- drift layer 2: z2 = relu(W2^T z1 + b2) ----
            ps2 = ppb.tile([128, SB], F32, name=f"ps2{s}", tag="ppb")
            for j in range(2):
                sl = slice(j * HB, (j + 1) * HB)
                nc.tensor.matmul(
                    ps2[:, sl], w2[:, :], z1[:, sl], start=True, stop=True
                )
            z2 = wk.tile([128, SB], BF16, name=f"z2{s}", tag=f"z2{s}")
            nc.scalar.activation(z2[:, 0:HB], ps2[:, 0:HB], AF.Relu, bias=b2[:])
            nc.vector.tensor_scalar(
                z2[:, HB:], ps2[:, HB:], b2[:], 0.0, ALU.add, ALU.max
            )

            # ---- drift out + h carry: ps3 = h + dt*(z2 @ dW3) ----
            # identity matmul first (h ready at step start), then the two
            # column-tiled bf16 halves accumulate (A -> rows 0:64 via (0,0),
            # B -> rows 64:128 via (0,64)).
            ps3 = pps.tile([128, HB], F32, name=f"ps3{s}", tag="pps")
            nc.tensor.matmul(
                ps3[:, :], idn[:, :], h_s[:, :],
                start=True, stop=False, skip_group_check=True,
            )
            nc.tensor.matmul(
                ps3[0:64, :], w3s[:, k, :], z2[:, 0:HB],
                start=False, stop=False, skip_group_check=True,
            )
            nc.tensor.matmul(
                ps3[64:128, :], w3s[:, k, :], z2[:, HB:],
                start=False, stop=True, tile_position=(0, 64),
                skip_group_check=True,
            )
            # h' = (ps3 + dt*db3) + sigmoid*zs
            h_new = hp.tile([128, HB], F32R, name=f"h{s}", tag=f"h{s}")
            nc.vector.scalar_tensor_tensor(
                h_new[:], ps3[:], dtb3[:, k : k + 1], tt[:], ALU.add, ALU.add
            )
            return h_new

        for k in range(STEPS):
            for s in range(2):
                h_cur[s] = step(k, s, h_cur[s])

        # ---- readout: out = relu(h @ rW1 + rb1) @ rW2 + rb2 ----
        h_unp = wk.tile([H, BC], F32R, name="h_unp", tag="h_unp")
        for s in range(2):
            nc.sync.dma_start(h_unp[:, s * SB : s * SB + HB], h_cur[s][0:64, :])
            nc.sync.dma_start(h_unp[:, s * SB + HB : (s + 1) * SB], h_cur[s][64:128, :])

        r1 = wk.tile([32, BC], BF16, name="r1", tag="r1")
        for q in range(4):
            sl = slice(q * HB, (q + 1) * HB)
            psr = pps.tile([128, HB], F32, name="psr", tag="pps")
            nc.tensor.matmul(
                psr[0:32, :], rw1[:, :], h_unp[:, sl], start=True, stop=True
            )
            nc.scalar.activation(r1[:, sl], psr[0:32, :], AF.Relu, bias=rb1[:])

        osb = wk.tile([2, BC], F32, name="osb", tag="osb")
        for q in range(4):
            sl = slice(q * HB, (q + 1) * HB)
            pso = pps.tile([128, HB], F32, name="pso", tag="pps")
            nc.tensor.matmul(
                pso[0:2, :], rw2[:, :], r1[:, sl], start=True, stop=True
            )
            nc.scalar.activation(osb[:, sl], pso[0:2, :], AF.Identity, bias=rb2[:])
        nc.sync.dma_start(d_out[:, :], osb[:])

    nc.compile()
    _CACHE["nc"] = nc
    return nc


def _dup(a, dt=NPBF):
    return np.ascontiguousarray(np.concatenate([a, a], axis=0).astype(dt))


def _blkdiag(a, dt=NPBF):
    n, m = a.shape
    out = np.zeros((2 * n, 2 * m), np.float32)
    out[:n, :m] = a
    out[n:, m:] = a
    return np.ascontiguousarray(out.astype(dt))


def _prep_in_maps(inputs):
    xp = np.asarray(inputs["x_path"], dtype=np.float32)
    t_span = np.asarray(inputs["t_span"], dtype=np.float32)
    dw = np.asarray(inputs["dW"], dtype=np.float32)

    Tm1 = np.int32(xp.shape[1] - 1)
    t_max = t_span[-1]
    idx = np.clip(
        (t_span[:-1] / t_max * np.float32(Tm1)).astype(np.int32), 0, Tm1
    )
    dts = (t_span[1:] - t_span[:-1]).astype(np.float32)
    sq = np.sqrt(dts).astype(np.float32)

    gscale = np.asarray(inputs["gscale"], dtype=np.float32)
    w1 = np.asarray(inputs["dW1"], dtype=np.float32)
    w2 = np.asarray(inputs["dW2"], dtype=np.float32)
    w3 = np.asarray(inputs["dW3"], dtype=np.float32)
    db1 = np.asarray(inputs["db1"], dtype=np.float32)
    db2 = np.asarray(inputs["db2"], dtype=np.float32)
    db3 = np.asarray(inputs["db3"], dtype=np.float32)
    gw1 = np.asarray(inputs["gW1"], dtype=np.float32)
    gw2 = np.asarray(inputs["gW2"], dtype=np.float32)
    gb1 = np.asarray(inputs["gb1"], dtype=np.float32)
    gb2 = np.asarray(inputs["gb2"], dtype=np.float32)
    rw1 = np.asarray(inputs["rW1"], dtype=np.float32)
    rb1 = np.asarray(inputs["rb1"], dtype=np.float32)
    rw2 = np.asarray(inputs["rW2"], dtype=np.float32)
    rb2 = np.asarray(inputs["rb2"], dtype=np.float32)

    w3s = w3[None, :, :] * dts[:, None, None]  # [STEPS, DW, H]

    common = {
        "w1h": _dup(w1[:H], np.float32),
        "w1x": _dup(w1[H:]),
        "w2": np.ascontiguousarray(w2.astype(NPBF)),
        "w3s": np.ascontiguousarray(w3s.astype(NPBF)),
        "gw1": _blkdiag(gw1, np.float32),
        "gw2": _blkdiag(gw2),
        "ident": np.eye(DW, dtype=np.float32),
        "b1": np.ascontiguousarray(db1.reshape(DW, 1)),
        "b2": np.ascontiguousarray(db2.reshape(DW, 1)),
        "dtb3": _dup((dts[:, None] * db3[None, :]).T, np.float32),  # [128, STEPS]
        "gb1": _dup(gb1.reshape(H, 1), np.float32),
        "gb2": _dup(gb2.reshape(H, 1), np.float32),
        "rw1": np.ascontiguousarray(rw1.astype(np.float32)),
        "rb1": np.ascontiguousarray(rb1.reshape(32, 1)),
        "rw2": np.ascontiguousarray(rw2.astype(NPBF)),
        "rb2": np.ascontiguousarray(rb2.reshape(2, 1)),
        "h0": np.zeros((128, HB), np.float32),
    }

    xg = xp[:, idx, :]  # [B, STEPS, F]
    zsc = gscale[None, :] * sq[:, None]  # [STEPS, F]

    in_maps = []
    for c in range(NCORES):
        rows = slice(c * BC, (c + 1) * BC)
        # (stream, half, b', k, f) -> (k, stream, half, f, b')
        xt = np.ascontiguousarray(
            xg[rows]
            .reshape(2, 2, HB, STEPS, FX)
            .transpose(3, 0, 1, 4, 2)
            .reshape(STEPS, 2, 128, HB)
            .astype(NPBF)
        )
        zc = dw[:, rows, :] * zsc[:, None, :]  # [STEPS, BC, H]
        zst = np.ascontiguousarray(
            zc.reshape(STEPS, 2, 2, HB, H)
            .transpose(0, 1, 2, 4, 3)
            .reshape(STEPS, 2, 128, HB)
            .astype(NPBF)
        )
        m = dict(common)
        m["xt"] = xt
        m["zst"] = zst
        in_maps.append(m)
    return in_maps


def kernel(**inputs):
    nc = _build()
    in_maps = _prep_in_maps(inputs)
    run_kwargs = dict(_CACHE.get("run_kwargs", {}))
    res = run_bass_kernel_spmd(nc, in_maps, list(range(NCORES)), **run_kwargs)
    _CACHE["last_results"] = res
    mu = np.concatenate([res.results[c]["out"][0] for c in range(NCORES)])
    ls = np.concatenate([res.results[c]["out"][1] for c in range(NCORES)])
    return mu, ls


# revision 16
# speedup vs baseline: 2.2324x; 2.1840x over previous
"""Trainium2 Bass kernel for the NSDE model (Euler-Maruyama scan + MLPs).

Strategy:
  - Data-parallel over batch: 16384 rows -> 8 cores x 2048 rows.
  - Only the 20 time slices of x_path that the scan actually reads are
    shipped to the device (indices computed on host from t_span).
  - Feature-major layout on chip: activations are [feature, batch] so every
    matmul uses the weight matrix directly as lhsT (out = W^T @ actT) and
    biases are per-partition scalars.
  - The per-core batch (2048) is processed as TWO independent interleaved
    streams of 1024 rows. The Euler-Maruyama scan is inherently serial, so a
    single stream leaves every engine idle most of the time; two streams
    keep the tensor/scalar/vector engines busy with the other stream while
    one waits on its dependency chain.
  - Within a stream, 64-feature tensors are "packed": partitions 0-63 hold
    features of its first 512 rows, partitions 64-127 the second 512. Drift
    layer1 uses PE row tiling (tile_position=(64,0), duplicated weights);
    diffusion layers use block-diagonal weights; drift output uses column
    tiling ((0,64) writes psum rows 64:128).
  - Bulk matmuls are bf16 (1 cycle/row). The h carry stays high precision:
    h is float32r (4-byte, ~1 cyc/row vs fp32's two half-rate passes) and
    is added into the drift-out PSUM exactly via a f32r identity matmul.
    h's consumers (drift l1 h-part, diffusion l1, identity) run f32r.
  - h' = (psum3 + dt*db3) + sigmoid*zs via one fused scalar_tensor_tensor.
  - dt and sqrt(dt) folds are done on host: dW3*dt per step, db3*dt, and
    the noise is pre-scaled zs = dW * gscale * sqrt(dt) (bf16).
  - Elementwise work is balanced across ScalarE/VectorE (both ~1 elem/cyc
    from PSUM) with the noise multiply on GpSimd.
"""

import os
from contextlib import ExitStack

import ml_dtypes
import numpy as np

import concourse.bass as bass
import concourse.mybir as mybir
import concourse.tile as tile
from concourse import bacc
from concourse.bass_utils import run_bass_kernel_spmd

F32 = mybir.dt.float32
F32R = mybir.dt.float32r
BF16 = mybir.dt.bfloat16
AF = mybir.ActivationFunctionType
ALU = mybir.AluOpType

NPBF = ml_dtypes.bfloat16

STEPS = 20
NCORES = 8
B = 16384
BC = B // NCORES  # per-core batch: 2048
SB = BC // 2  # per-stream batch: 1024
HB = SB // 2  # packed free size per stream: 512
H = 64
FX = 64
DW = 128

_CACHE = {}


def _build():
    if "nc" in _CACHE:
        return _CACHE["nc"]

    nc = bacc.Bacc("TRN2", target_bir_lowering=False, debug=False)

    def din(name, shape, dt=F32):
        return nc.dram_tensor(name, shape, dt, kind="ExternalInput")

    d_xt = din("xt", [STEPS, 2, 128, HB], BF16)  # [step, stream, feat-packed, b]
    d_zst = din("zst", [STEPS, 2, 128, HB], BF16)
    d_w1h = din("w1h", [128, DW], F32R)  # dW1[:64] duplicated (f32r: reads h)
    d_w1x = din("w1x", [128, DW], BF16)  # dW1[64:] duplicated
    d_w2 = din("w2", [DW, DW], BF16)
    d_w3s = din("w3s", [STEPS, DW, H], BF16)  # dW3 * dt_k
    d_gw1 = din("gw1", [128, DW], F32R)  # blockdiag(gW1, gW1) (f32r: reads h)
    d_gw2 = din("gw2", [DW, DW], BF16)  # blockdiag(gW2, gW2)
    d_id = din("ident", [128, DW], F32R)  # 128x128 identity (h carry)
    d_b1 = din("b1", [DW, 1])
    d_b2 = din("b2", [DW, 1])
    d_dtb3 = din("dtb3", [128, STEPS])  # dt_k * db3, packed-dup
    d_gb1 = din("gb1", [128, 1])
    d_gb2 = din("gb2", [128, 1])
    d_rw1 = din("rw1", [H, 32], F32R)
    d_rb1 = din("rb1", [32, 1])
    d_rw2 = din("rw2", [32, 2], BF16)
    d_rb2 = din("rb2", [2, 1])
    d_h0 = din("h0", [128, HB], F32R)  # zeros (f32r memset unsupported)
    d_out = nc.dram_tensor("out", [2, BC], F32, kind="ExternalOutput")

    with ExitStack() as ctx:
        tc = ctx.enter_context(tile.TileContext(nc))
        consts = ctx.enter_context(tc.tile_pool(name="consts", bufs=1))
        xzp = ctx.enter_context(tc.tile_pool(name="xzp", bufs=3))
        hp = ctx.enter_context(tc.tile_pool(name="hp", bufs=2))
        wk = ctx.enter_context(tc.tile_pool(name="wk", bufs=2))
        ppb = ctx.enter_context(tc.tile_pool(name="ppb", bufs=4, space="PSUM"))
        pps = ctx.enter_context(tc.tile_pool(name="pps", bufs=4, space="PSUM"))

        def cload(dram_ap, shape, name, dt=F32):
            t = consts.tile(shape, dt, name=name, tag=name)
            nc.sync.dma_start(t[:], dram_ap)
            return t

        w1h = cload(d_w1h[:, :], [128, DW], "w1h", F32R)
        w1x = cload(d_w1x[:, :], [128, DW], "w1x", BF16)
        w2 = cload(d_w2[:, :], [DW, DW], "w2", BF16)
        w3s = cload(
            d_w3s[:, :, :].rearrange("k p m -> p k m"), [DW, STEPS, H], "w3s", BF16
        )
        gw1 = cload(d_gw1[:, :], [128, DW], "gw1", F32R)
        gw2 = cload(d_gw2[:, :], [DW, DW], "gw2", BF16)
        idn = cload(d_id[:, :], [128, DW], "idn", F32R)
        b1 = cload(d_b1[:, :], [DW, 1], "b1")
        b2 = cload(d_b2[:, :], [DW, 1], "b2")
        dtb3 = cload(d_dtb3[:, :], [128, STEPS], "dtb3")
        gb1 = cload(d_gb1[:, :], [128, 1], "gb1")
        gb2 = cload(d_gb2[:, :], [128, 1], "gb2")
        rw1 = cload(d_rw1[:, :], [H, 32], "rw1", F32R)
        rb1 = cload(d_rb1[:, :], [32, 1], "rb1")
        rw2 = cload(d_rw2[:, :], [32, 2], "rw2", BF16)
        rb2 = cload(d_rb2[:, :], [2, 1], "rb2")

        h_cur = []
        for s in range(2):
            h0 = hp.tile([128, HB], F32R, name=f"h{s}", tag=f"h{s}")
            nc.sync.dma_start(h0[:], d_h0[:, :])
            h_cur.append(h0)

        def step_pair(k):
            """One Euler-Maruyama step for both streams, stage-interleaved
            so consecutive PE instructions belong to alternating streams
            (each one's dependency was satisfied while the other ran)."""
            xk, zk, g1, sg, tt, z1, z2 = {}, {}, {}, {}, {}, {}, {}
            psg, pss, ps1, ps2, ps3 = {}, {}, {}, {}, {}

            for s in range(2):
                xk[s] = xzp.tile([128, HB], BF16, name=f"xk{s}", tag=f"xk{s}")
                nc.sync.dma_start(xk[s][:], d_xt[k, s])
                zk[s] = xzp.tile([128, HB], BF16, name=f"zk{s}", tag=f"zk{s}")
                nc.sync.dma_start(zk[s][:], d_zst[k, s])

            # ---- diffusion layer 1 (blockdiag weights) ----
            for s in range(2):
                psg[s] = pps.tile([128, HB], F32, name=f"psg{s}", tag="pps")
                nc.tensor.matmul(
                    psg[s][:, :], gw1[:, :], h_cur[s][:, :], start=True, stop=True
                )
            for s in range(2):
                g1[s] = wk.tile([128, HB], BF16, name=f"g1{s}", tag=f"g1{s}")
                if s == 0:
                    nc.scalar.activation(g1[s][:], psg[s][:], AF.Relu, bias=gb1[:])
                else:
                    nc.vector.tensor_scalar(
                        g1[s][:], psg[s][:], gb1[:], 0.0, ALU.add, ALU.max
                    )

            # ---- drift layer 1 matmuls (row-tiled) ----
            for s in range(2):
                ps1[s] = [
                    ppb.tile([128, HB], F32, name=f"ps1{s}{j}", tag="ppb")
                    for j in range(2)
                ]
                for j, tp in ((0, None), (1, (64, 0))):
                    lo, hi = 64 * j, 64 * (j + 1)
                    nc.tensor.matmul(
                        ps1[s][j][:, :], w1h[lo:hi, :], h_cur[s][lo:hi, :],
                        start=True, stop=False, tile_position=tp,
                    )
                    nc.tensor.matmul(
                        ps1[s][j][:, :], w1x[lo:hi, :], xk[s][lo:hi, :],
                        start=False, stop=True, tile_position=tp,
                    )

            # ---- diffusion layer 2 + sigmoid + noise ----
            for s in range(2):
                pss[s] = pps.tile([128, HB], F32, name=f"pss{s}", tag="pps")
                nc.tensor.matmul(
                    pss[s][:, :], gw2[:, :], g1[s][:, :], start=True, stop=True
                )
            for s in range(2):
                sg[s] = wk.tile([128, HB], BF16, name=f"sg{s}", tag=f"sg{s}")
                nc.scalar.activation(sg[s][:], pss[s][:], AF.Sigmoid, bias=gb2[:])
                tt[s] = wk.tile([128, HB], BF16, name=f"tt{s}", tag=f"tt{s}")
                nc.gpsimd.tensor_mul(tt[s][:], sg[s][:], zk[s][:])

            # ---- drift layer 1 relu (split ACT/DVE) ----
            for s in range(2):
                z1[s] = wk.tile([128, SB], BF16, name=f"z1{s}", tag=f"z1{s}")
                nc.scalar.activation(
                    z1[s][:, 0:HB], ps1[s][0][:], AF.Relu, bias=b1[:]
                )
                nc.vector.tensor_scalar(
                    z1[s][:, HB:], ps1[s][1][:], b1[:], 0.0, ALU.add, ALU.max
                )

            # ---- drift layer 2 ----
            for s in range(2):
                ps2[s] = [
                    ppb.tile([128, HB], F32, name=f"ps2{s}{j}", tag="ppb")
                    for j in range(2)
                ]
                for j in range(2):
                    nc.tensor.matmul(
                        ps2[s][j][:, :], w2[:, :], z1[s][:, j * HB : (j + 1) * HB],
                        start=True, stop=True,
                    )
            for s in range(2):
                z2[s] = wk.tile([128, SB], BF16, name=f"z2{s}", tag=f"z2{s}")
                nc.scalar.activation(
                    z2[s][:, 0:HB], ps2[s][0][:], AF.Relu, bias=b2[:]
                )
                nc.vector.tensor_scalar(
                    z2[s][:, HB:], ps2[s][1][:], b2[:], 0.0, ALU.add, ALU.max
                )

            # ---- drift out + h carry: ps3 = h + dt*(z2 @ dW3) ----
            for s in range(2):
                ps3[s] = pps.tile([128, HB], F32, name=f"ps3{s}", tag="pps")
                nc.tensor.matmul(
                    ps3[s][:, :], idn[:, :], h_cur[s][:, :],
                    start=True, stop=False, skip_group_check=True,
                )
            for s in range(2):
                nc.tensor.matmul(
                    ps3[s][0:64, :], w3s[:, k, :], z2[s][:, 0:HB],
                    start=False, stop=False, skip_group_check=True,
                )
                nc.tensor.matmul(
                    ps3[s][64:128, :], w3s[:, k, :], z2[s][:, HB:],
                    start=False, stop=True, tile_position=(0, 64),
                    skip_group_check=True,
                )
            # h' = (ps3 + dt*db3) + sigmoid*zs
            for s in range(2):
                h_new = hp.tile([128, HB], F32R, name=f"h{s}", tag=f"h{s}")
                nc.vector.scalar_tensor_tensor(
                    h_new[:], ps3[s][:], dtb3[:, k : k + 1], tt[s][:],
                    ALU.add, ALU.add,
                )
                h_cur[s] = h_new

        for k in range(STEPS):
            step_pair(k)

        # ---- readout: out = relu(h @ rW1 + rb1) @ rW2 + rb2 ----
        h_unp = wk.tile([H, BC], F32R, name="h_unp", tag="h_unp")
        for s in range(2):
            nc.sync.dma_start(h_unp[:, s * SB : s * SB + HB], h_cur[s][0:64, :])
            nc.sync.dma_start(h_unp[:, s * SB + HB : (s + 1) * SB], h_cur[s][64:128, :])

        r1 = wk.tile([32, BC], BF16, name="r1", tag="r1")
        for q in range(4):
            sl = slice(q * HB, (q + 1) * HB)
            psr = pps.tile([128, HB], F32, name="psr", tag="pps")
            nc.tensor.matmul(
                psr[0:32, :], rw1[:, :], h_unp[:, sl], start=True, stop=True
            )
            nc.scalar.activation(r1[:, sl], psr[0:32, :], AF.Relu, bias=rb1[:])

        osb = wk.tile([2, BC], F32, name="osb", tag="osb")
        for q in range(4):
            sl = slice(q * HB, (q + 1) * HB)
            pso = pps.tile([128, HB], F32, name="pso", tag="pps")
            nc.tensor.matmul(
                pso[0:2, :], rw2[:, :], r1[:, sl], start=True, stop=True
            )
            nc.scalar.activation(osb[:, sl], pso[0:2, :], AF.Identity, bias=rb2[:])
        nc.sync.dma_start(d_out[:, :], osb[:])

    nc.compile()
    _CACHE["nc"] = nc
    return nc


def _dup(a, dt=NPBF):
    return np.ascontiguousarray(np.concatenate([a, a], axis=0).astype(dt))


def _blkdiag(a, dt=NPBF):
    n, m = a.shape
    out = np.zeros((2 * n, 2 * m), np.float32)
    out[:n, :m] = a
    out[n:, m:] = a
    return np.ascontiguousarray(out.astype(dt))


def _prep_in_maps(inputs):
    xp = np.asarray(inputs["x_path"], dtype=np.float32)
    t_span = np.asarray(inputs["t_span"], dtype=np.float32)
    dw = np.asarray(inputs["dW"], dtype=np.float32)

    Tm1 = np.int32(xp.shape[1] - 1)
    t_max = t_span[-1]
    idx = np.clip(
        (t_span[:-1] / t_max * np.float32(Tm1)).astype(np.int32), 0, Tm1
    )
    dts = (t_span[1:] - t_span[:-1]).astype(np.float32)
    sq = np.sqrt(dts).astype(np.float32)

    gscale = np.asarray(inputs["gscale"], dtype=np.float32)
    w1 = np.asarray(inputs["dW1"], dtype=np.float32)
    w2 = np.asarray(inputs["dW2"], dtype=np.float32)
    w3 = np.asarray(inputs["dW3"], dtype=np.float32)
    db1 = np.asarray(inputs["db1"], dtype=np.float32)
    db2 = np.asarray(inputs["db2"], dtype=np.float32)
    db3 = np.asarray(inputs["db3"], dtype=np.float32)
    gw1 = np.asarray(inputs["gW1"], dtype=np.float32)
    gw2 = np.asarray(inputs["gW2"], dtype=np.float32)
    gb1 = np.asarray(inputs["gb1"], dtype=np.float32)
    gb2 = np.asarray(inputs["gb2"], dtype=np.float32)
    rw1 = np.asarray(inputs["rW1"], dtype=np.float32)
    rb1 = np.asarray(inputs["rb1"], dtype=np.float32)
    rw2 = np.asarray(inputs["rW2"], dtype=np.float32)
    rb2 = np.asarray(inputs["rb2"], dtype=np.float32)

    w3s = w3[None, :, :] * dts[:, None, None]  # [STEPS, DW, H]

    common = {
        "w1h": _dup(w1[:H], np.float32),
        "w1x": _dup(w1[H:]),
        "w2": np.ascontiguousarray(w2.astype(NPBF)),
        "w3s": np.ascontiguousarray(w3s.astype(NPBF)),
        "gw1": _blkdiag(gw1, np.float32),
        "gw2": _blkdiag(gw2),
        "ident": np.eye(DW, dtype=np.float32),
        "b1": np.ascontiguousarray(db1.reshape(DW, 1)),
        "b2": np.ascontiguousarray(db2.reshape(DW, 1)),
        "dtb3": _dup((dts[:, None] * db3[None, :]).T, np.float32),  # [128, STEPS]
        "gb1": _dup(gb1.reshape(H, 1), np.float32),
        "gb2": _dup(gb2.reshape(H, 1), np.float32),
        "rw1": np.ascontiguousarray(rw1.astype(np.float32)),
        "rb1": np.ascontiguousarray(rb1.reshape(32, 1)),
        "rw2": np.ascontiguousarray(rw2.astype(NPBF)),
        "rb2": np.ascontiguousarray(rb2.reshape(2, 1)),
        "h0": np.zeros((128, HB), np.float32),
    }

    xg = xp[:, idx, :]  # [B, STEPS, F]
    zsc = gscale[None, :] * sq[:, None]  # [STEPS, F]

    in_maps = []
    for c in range(NCORES):
        rows = slice(c * BC, (c + 1) * BC)
        # (stream, half, b', k, f) -> (k, stream, half, f, b')
        xt = np.ascontiguousarray(
            xg[rows]
            .reshape(2, 2, HB, STEPS, FX)
            .transpose(3, 0, 1, 4, 2)
            .reshape(STEPS, 2, 128, HB)
            .astype(NPBF)
        )
        zc = dw[:, rows, :] * zsc[:, None, :]  # [STEPS, BC, H]
        zst = np.ascontiguousarray(
            zc.reshape(STEPS, 2, 2, HB, H)
            .transpose(0, 1, 2, 4, 3)
            .reshape(STEPS, 2, 128, HB)
            .astype(NPBF)
        )
        m = dict(common)
        m["xt"] = xt
        m["zst"] = zst
        in_maps.append(m)
    return in_maps


def kernel(**inputs):
    nc = _build()
    in_maps = _prep_in_maps(inputs)
    run_kwargs = dict(_CACHE.get("run_kwargs", {}))
    res = run_bass_kernel_spmd(nc, in_maps, list(range(NCORES)), **run_kwargs)
    _CACHE["last_results"] = res
    mu = np.concatenate([res.results[c]["out"][0] for c in range(NCORES)])
    ls = np.concatenate([res.results[c]["out"][1] for c in range(NCORES)])
    return mu, ls
